# revision 1
# baseline (speedup 1.0000x reference)
# Cross-attention kernel for Trainium2, 8 NeuronCores.
#
# Sharding: data-parallel over (batch, query-half): core = 2*b + half handles
# batch b, queries [half*1024, (half+1)*1024). No collectives.
#
# On-device layout is feature-major ("transposed"): activations live as
# [feature, token]. The host pre-transposes inputs and post-transposes the
# output. Both layernorms are folded into the projections:
#   LN(x) @ W.T = (x*r) @ W'.T - (m*r) x S + bq, with W' = (W*g).T,
#   S[j] = sum_d W'[d,j], bq = W @ b, applied via a K=2 auxiliary matmul
#   with moving rows [m*r; 1].
# Attention runs with keys on partitions (logits transposed), so softmax
# denominators come from an all-ones column appended to V (M=65 matmul
# output row 64 = sum of exp). Max-subtraction is skipped: |logits/temp| < 3.
import os
import sys
import tempfile

# The neuron compile cache keys on the jax module hash, which does not cover
# the embedded Bass program — a stale NEFF can be silently reused. Use a
# fresh cache dir per process.
os.environ["NEURON_COMPILE_CACHE_URL"] = tempfile.mkdtemp(prefix="neff_cache_")
# The axon cassette (compile record/replay) fingerprints the module interface
# but not the embedded Bass program; salt it per process so edits always
# recompile instead of replaying a stale executable.
os.environ["AXON_CASSETTE_SALT"] = f"ca-{os.getpid()}-{os.urandom(4).hex()}"

for _p in ("/opt/trn_rl_repo",):
    if os.path.isdir(_p) and _p not in sys.path:
        sys.path.insert(0, _p)

import numpy as np
import ml_dtypes
from contextlib import ExitStack

import concourse.bass as bass
import concourse.tile as tile
from concourse import bacc, mybir
from concourse.bass_utils import run_bass_kernel_spmd

F32 = mybir.dt.float32
F32R = mybir.dt.float32r
BF16 = mybir.dt.bfloat16
AF = mybir.ActivationFunctionType

B, NQ, NK, D = 4, 2048, 2048, 512
H, DH = 8, 64
NQS = NQ // 2  # queries per core
TEMP = float(np.sqrt(512.0))
LN_EPS = 1e-5
N_CORES = 8

_CACHE = {}
# Interface salt: the remote executable cache fingerprints the module
# interface but not the embedded program; bump per kernel edit to force
# recompilation.
SALT = "v09"
SALT_N = 9


def _build_program():
    nc = bacc.Bacc("TRN2", target_bir_lowering=False, debug=False)

    def din(name, shape, dt=F32R):
        return nc.dram_tensor(f"{name}_{SALT}", shape, dt,
                              kind="ExternalInput").ap()

    qt_d = din("qt", [128, 4, NQS], BF16)
    kt_d = din("kt", [128, 4, NK], BF16)
    vt_d = din("vt", [128, 4, NK], BF16)
    wq_d = din("wq", [128, 4, D], BF16)
    wk_d = din("wk", [128, 4, D], BF16)
    wv_d = din("wv", [128, 4, D], BF16)
    wo_d = din("wo", [128, 4, D])
    aq_d = din("aq", [2, D], BF16)
    ak_d = din("ak", [2, D], BF16)
    ao_d = din("ao", [2, D])
    gb_d = din("gb", [128, 2, 4], F32)
    salt_d = din("salt", [1, 8 + SALT_N], F32)
    out_d = nc.dram_tensor(f"out_{SALT}", [128, 4, NQS], F32,
                           kind="ExternalOutput").ap()

    with tile.TileContext(nc) as tc, ExitStack() as top:
        persist = top.enter_context(tc.tile_pool(name="persist", bufs=1))
        qTs = persist.tile([128, 4, NQS], BF16)       # projected q, feature-major
        kTs = persist.tile([128, 4, NK], BF16)        # projected k
        vaug = persist.tile([128, 16, 8, 65], BF16)   # v natural + ones col per head
        oTs = persist.tile([128, 4, NQS], F32R)       # normalized attention out
        wo_sb = persist.tile([128, 4, D], F32R)
        ao_sb = persist.tile([2, D], F32R)
        gb_sb = persist.tile([128, 2, 4], F32)
        ones32 = persist.tile([128, NK], F32)
        onesr = persist.tile([128, 128], F32R)
        onesb = persist.tile([128, 128], BF16)
        ones_rb = persist.tile([1, NK], BF16)
        eps_t = persist.tile([128, 1], F32)

        nc.sync.dma_start(out=wo_sb, in_=wo_d)
        nc.sync.dma_start(out=ao_sb, in_=ao_d)
        nc.sync.dma_start(out=gb_sb, in_=gb_d)
        nc.vector.memset(ones32, 1.0)
        nc.sync.dma_start(out=eps_t, in_=salt_d[0:1, 0:1].to_broadcast([128, 1]))
        nc.vector.tensor_copy(onesr, ones32[:, 0:128])
        nc.vector.memset(onesb, 1.0)
        nc.vector.memset(ones_rb, 1.0)
        ones_r = persist.tile([1, NK], F32R)
        nc.vector.tensor_copy(ones_r, ones32[0:1, :])
        # ones columns of vaug (slot 64 of each head's lhsT)
        nc.vector.memset(vaug[:, :, :, 64], 1.0)

        def ln_stats_prescale(xin, nchunks, st_ps, work, aux):
            """Column LN stats of xin [128, 4, nchunks*512]; prescales xin by
            r in place; fills aux [2, nchunks*512] rows with [m*r; 1]."""
            bf = xin.dtype == BF16
            o_mm = onesb if bf else onesr
            nc.sync.dma_start(
                out=aux[1:2, :],
                in_=(ones_rb if bf else ones_r)[0:1, 0 : nchunks * 512])
            for n2 in range(nchunks):
                ns = slice(512 * n2, 512 * n2 + 512)
                ps_sum = st_ps.tile([128, 512], F32, name="ps_sum")
                ps_ssq = st_ps.tile([128, 512], F32, name="ps_ssq")
                for jc in range(4):
                    nc.tensor.matmul(ps_sum, o_mm, xin[:, jc, ns],
                                     start=(jc == 0), stop=(jc == 3))
                for jc in range(4):
                    sq = work.tile([128, 512], BF16 if bf else F32R,
                                   name="sq", bufs=3)
                    with nc.allow_low_precision("f32r keeps fp32 storage"):
                        nc.vector.tensor_mul(sq, xin[:, jc, ns],
                                             xin[:, jc, ns])
                    nc.tensor.matmul(ps_ssq, o_mm, sq,
                                     start=(jc == 0), stop=(jc == 3))
                # replicated stats rows: m = sum/512 ; var = (ssq - sum*m)/512
                m_b = work.tile([128, 512], F32, name="m_b", bufs=2)
                nc.scalar.mul(m_b, ps_sum, 1.0 / 512.0)
                t2 = work.tile([128, 512], F32, name="t2", bufs=2)
                nc.vector.tensor_mul(t2, m_b, ps_sum)
                dv = work.tile([128, 512], F32, name="dv", bufs=2)
                nc.vector.tensor_sub(dv, ps_ssq, t2)
                std = work.tile([128, 512], F32, name="std", bufs=2)
                nc.scalar.activation(std, dv, AF.Sqrt, bias=eps_t,
                                     scale=1.0 / 512.0)
                r_b = work.tile([128, 512], F32, name="r_b", bufs=2)
                nc.vector.reciprocal(r_b, std)
                mr_b = work.tile([128, 512], F32R, name="mr_b", bufs=2)
                with nc.allow_low_precision("f32r keeps fp32 storage"):
                    nc.vector.tensor_mul(mr_b, m_b, r_b)
                nc.vector.tensor_copy(aux[0:1, ns], mr_b[0:1, :])
                for jc in range(4):
                    with nc.allow_low_precision("f32r keeps fp32 storage"):
                        nc.vector.tensor_mul(xin[:, jc, ns], xin[:, jc, ns], r_b)

        def project(dst, xin, w_sb, aux_lhs, aux, nchunks, mm_ps, pool):
            """dst[:, jc, n] = sum_kc w_sb[:,kc,jcblk].T @ xin[:,kc,n] + aux."""
            for jc in range(4):
                js = slice(128 * jc, 128 * jc + 128)
                pmms = [mm_ps.tile([128, 512], F32, name=f"pmm{n2}",
                                   bufs=1)
                        for n2 in range(nchunks)]
                for kc in range(4):
                    for n2 in range(nchunks):
                        ns = slice(512 * n2, 512 * n2 + 512)
                        nc.tensor.matmul(pmms[n2], w_sb[:, kc, js],
                                         xin[:, kc, ns],
                                         start=(kc == 0), stop=False)
                for n2 in range(nchunks):
                    ns = slice(512 * n2, 512 * n2 + 512)
                    nc.tensor.matmul(pmms[n2], aux_lhs[:, js], aux[:, ns],
                                     start=False, stop=True)
                    nc.vector.tensor_copy(dst[:, jc, ns], pmms[n2])

        # ---- Q phase: LN + projection ----
        with tc.tile_pool(name="wq_p", bufs=1) as wq_p, \
             tc.tile_pool(name="q_sb", bufs=1) as q_sb, \
             tc.tile_pool(name="q_st", bufs=1, space="PSUM") as q_st, \
             tc.tile_pool(name="q_mm", bufs=1, space="PSUM") as q_mm:
            wq_sb = wq_p.tile([128, 4, D], BF16)
            nc.sync.dma_start(out=wq_sb, in_=wq_d)
            aq_sb = q_sb.tile([2, D], BF16)
            nc.sync.dma_start(out=aq_sb, in_=aq_d)
            qtin = q_sb.tile([128, 4, NQS], BF16)
            nc.sync.dma_start(out=qtin, in_=qt_d)
            auxq = q_sb.tile([2, NQS], BF16)
            ln_stats_prescale(qtin, 2, q_st, q_sb, auxq)
            project(qTs, qtin, wq_sb, aq_sb, auxq, 2, q_mm, q_sb)

        # ---- K phase ----
        with tc.tile_pool(name="wk_p", bufs=1) as wk_p, \
             tc.tile_pool(name="k_sb", bufs=1) as k_sb, \
             tc.tile_pool(name="k_st", bufs=1, space="PSUM") as k_st, \
             tc.tile_pool(name="k_mm", bufs=1, space="PSUM") as k_mm:
            wk_sb = wk_p.tile([128, 4, D], BF16)
            nc.sync.dma_start(out=wk_sb, in_=wk_d)
            ak_sb = k_sb.tile([2, D], BF16)
            nc.sync.dma_start(out=ak_sb, in_=ak_d)
            ktin = k_sb.tile([128, 4, NK], BF16)
            nc.sync.dma_start(out=ktin, in_=kt_d)
            auxk = k_sb.tile([2, NK], BF16)
            ln_stats_prescale(ktin, 4, k_st, k_sb, auxk)
            project(kTs, ktin, wk_sb, ak_sb, auxk, 4, k_mm, k_sb)

        # ---- V phase: plain projection into natural layout + ones col ----
        with tc.tile_pool(name="wv_p", bufs=1) as wv_p, \
             tc.tile_pool(name="v_sb", bufs=1) as v_sb, \
             tc.tile_pool(name="v_mm", bufs=1, space="PSUM") as v_mm:
            wv_sb = wv_p.tile([128, 4, D], BF16)
            nc.sync.dma_start(out=wv_sb, in_=wv_d)
            vtin = v_sb.tile([128, 4, NK], BF16)
            nc.sync.dma_start(out=vtin, in_=vt_d)
            for t in range(16):
                ts = slice(128 * t, 128 * t + 128)
                pv = v_mm.tile([128, 512], F32, name="pv", bufs=3)
                for kc in range(4):
                    nc.tensor.matmul(pv, vtin[:, kc, ts], wv_sb[:, kc, :],
                                     start=(kc == 0), stop=(kc == 3))
                nc.vector.tensor_copy(
                    vaug[:, t, :, 0:64],
                    pv.rearrange("p (h v) -> p h v", h=8))

        # ---- Attention: per head, streaming over key chunks ----
        with tc.tile_pool(name="at_sb", bufs=1) as at_sb, \
             tc.tile_pool(name="at_L", bufs=1, space="PSUM") as at_L, \
             tc.tile_pool(name="at_O", bufs=1, space="PSUM") as at_O:
            # Head pairs (2P, 2P+1) share feature chunk jc=P at partition rows
            # [0:64] / [64:128]: their K=64 L-matmuls row-tile into disjoint
            # halves of the PE array and execute concurrently. Emission is
            # software-pipelined: the L-matmuls of step p+1 enter the PE queue
            # before the O-matmuls of step p (which wait on exp(p)).
            def emit_L_pair(P, p):
                ks = slice(128 * p, 128 * p + 128)
                tiles = [at_L.tile([128, 1024], F32, name=f"psL{hh}", bufs=1)
                         for hh in range(2)]
                for c in range(2):
                    for hh in range(2):
                        rb = 64 * hh
                        nc.tensor.matmul(
                            tiles[hh][:, 512 * c : 512 * c + 512],
                            kTs[rb : rb + 64, P, ks],
                            qTs[rb : rb + 64, P, 512 * c : 512 * c + 512],
                            start=True, stop=True)
                return tiles

            for P in range(4):
                ps_o = [[at_O.tile([65, 512], F32, name=f"ps_o{hh}{c}",
                                   bufs=1) for c in range(2)]
                        for hh in range(2)]
                if P == 0:
                    psL_next = emit_L_pair(0, 0)
                for p in range(16):
                    psL = psL_next
                    exs = []
                    for hh in range(2):
                        ex = at_sb.tile([128, 1024], BF16, name=f"ex{hh}",
                                        bufs=4)
                        nc.scalar.activation(ex, psL[hh], AF.Exp,
                                             scale=1.0 / TEMP)
                        exs.append(ex)
                    if p < 15:
                        psL_next = emit_L_pair(P, p + 1)
                    elif P < 3:
                        psL_next = emit_L_pair(P + 1, 0)
                    for hh in range(2):
                        for c in range(2):
                            nc.tensor.matmul(
                                ps_o[hh][c], vaug[:, p, 2 * P + hh, :],
                                exs[hh][:, 512 * c : 512 * c + 512],
                                start=(p == 0), stop=(p == 15))
                for hh in range(2):
                    rb = 64 * hh
                    for c in range(2):
                        cs = slice(512 * c, 512 * c + 512)
                        rr = at_sb.tile([65, 512], F32R, name="rr", bufs=2)
                        with nc.allow_low_precision("f32r keeps fp32 storage"):
                            nc.vector.reciprocal(rr[64:65, :],
                                                 ps_o[hh][c][64:65, :])
                        pb = at_L.tile([64, 512], F32, name="psL0", bufs=1)
                        nc.tensor.matmul(pb, onesr[64:65, 0:64], rr[64:65, :],
                                         start=True, stop=True)
                        rbt = at_sb.tile([64, 512], F32, name="rbt", bufs=2)
                        nc.vector.tensor_copy(rbt, pb)
                        ost = at_sb.tile([64, 512], F32R, name="ost", bufs=2)
                        with nc.allow_low_precision("f32r keeps fp32 storage"):
                            nc.vector.tensor_mul(ost, ps_o[hh][c][0:64, :], rbt)
                        nc.sync.dma_start(out=oTs[rb : rb + 64, P, cs],
                                          in_=ost)

        # ---- Finale: LN fold + Wo + gelu + residual ----
        with tc.tile_pool(name="f_sb", bufs=1) as f_sb, \
             tc.tile_pool(name="f_st", bufs=1, space="PSUM") as f_st, \
             tc.tile_pool(name="f_mm", bufs=1, space="PSUM") as f_mm:
            auxo = f_sb.tile([2, NQS], F32R)
            ln_stats_prescale(oTs, 2, f_st, f_sb, auxo)
            mrbs = []
            for n2 in range(2):
                ns = slice(512 * n2, 512 * n2 + 512)
                pbm = f_mm.tile([128, 512], F32, name="pbm", bufs=2)
                nc.tensor.matmul(pbm, onesr[0:1, :], auxo[0:1, ns],
                                 start=True, stop=True)
                mrb = f_sb.tile([128, 512], F32, name=f"mrb{n2}", bufs=1)
                nc.vector.tensor_copy(mrb, pbm)
                mrbs.append(mrb)
            for jc in range(4):
                js = slice(128 * jc, 128 * jc + 128)
                for n2 in range(2):
                    ns = slice(512 * n2, 512 * n2 + 512)
                    mrb = mrbs[n2]
                    pg = f_mm.tile([128, 512], F32, name="pg", bufs=2)
                    for kc in range(4):
                        nc.tensor.matmul(pg, wo_sb[:, kc, js], oTs[:, kc, ns],
                                         start=(kc == 0), stop=False)
                    nc.tensor.matmul(pg, ao_sb[:, js], auxo[:, ns],
                                     start=False, stop=True)
                    gl = f_sb.tile([128, 512], F32, name="gl", bufs=2)
                    nc.scalar.activation(gl, pg, AF.Gelu)
                    # residual: (oTs - mr)*g + b   (oTs already prescaled by r)
                    ut = f_sb.tile([128, 512], F32, name="ut", bufs=2)
                    nc.vector.tensor_sub(ut, oTs[:, jc, ns], mrb)
                    nc.vector.tensor_scalar(
                        ut, ut, gb_sb[:, 0, jc : jc + 1],
                        gb_sb[:, 1, jc : jc + 1],
                        op0=mybir.AluOpType.mult, op1=mybir.AluOpType.add)
                    of = f_sb.tile([128, 512], F32, name="of", bufs=2)
                    nc.vector.tensor_add(of, ut, gl)
                    nc.sync.dma_start(out=out_d[:, jc, ns], in_=of)

    nc.compile()
    return nc


def _chunk_fm(x):
    """[512, N] feature-major -> [128, 4, N] (partition, chunk, col)."""
    n = x.shape[1]
    return np.ascontiguousarray(x.reshape(4, 128, n).transpose(1, 0, 2))


def _prep_inputs(Q, K, V, Wq, Wk, Wv, Wo, g, b, go, bo):
    WqT = np.ascontiguousarray((Wq * g[None, :]).T)
    WkT = np.ascontiguousarray((Wk * g[None, :]).T)
    WvT = np.ascontiguousarray(Wv.T)
    WoT = np.ascontiguousarray((Wo * go[None, :]).T)
    b16 = ml_dtypes.bfloat16
    shared = {
        f"wq_{SALT}": _chunk_fm(WqT).astype(b16),
        f"wk_{SALT}": _chunk_fm(WkT).astype(b16),
        f"wv_{SALT}": _chunk_fm(WvT).astype(b16),
        f"wo_{SALT}": _chunk_fm(WoT),
        f"aq_{SALT}": np.stack([-WqT.sum(0), Wq @ b]).astype(b16),
        f"ak_{SALT}": np.stack([-WkT.sum(0), Wk @ b]).astype(b16),
        f"ao_{SALT}": np.ascontiguousarray(np.stack([-WoT.sum(0), Wo @ bo])),
        f"gb_{SALT}": np.ascontiguousarray(
            np.stack([go.reshape(4, 128).T, bo.reshape(4, 128).T], axis=1)),
    }
    in_maps = []
    for core in range(N_CORES):
        bi, half = core // 2, core % 2
        qs = slice(half * NQS, (half + 1) * NQS)
        m = dict(shared)
        m[f"salt_{SALT}"] = np.full((1, 8 + SALT_N), LN_EPS, np.float32)
        m[f"qt_{SALT}"] = _chunk_fm(np.ascontiguousarray(Q[bi, qs, :].T)).astype(b16)
        m[f"kt_{SALT}"] = _chunk_fm(np.ascontiguousarray(K[bi].T)).astype(b16)
        m[f"vt_{SALT}"] = _chunk_fm(np.ascontiguousarray(V[bi].T)).astype(b16)
        in_maps.append(m)
    return in_maps


def kernel(Q, K, V, Wq, Wk, Wv, Wo, ln_qk_g, ln_qk_b, ln_o_g, ln_o_b,
           _trace=False):
    args = [np.asarray(a, dtype=np.float32) for a in
            (Q, K, V, Wq, Wk, Wv, Wo, ln_qk_g, ln_qk_b, ln_o_g, ln_o_b)]
    if "nc" not in _CACHE:
        _CACHE["nc"] = _build_program()
    nc = _CACHE["nc"]
    in_maps = _prep_inputs(*args)
    res = run_bass_kernel_spmd(nc, in_maps, core_ids=list(range(N_CORES)),
                               trace=_trace)
    _CACHE["last_results"] = res
    out = np.empty((B, NQ, D), dtype=np.float32)
    for core in range(N_CORES):
        bi, half = core // 2, core % 2
        o = res.results[core][f"out_{SALT}"]  # [128, 4, NQS]
        out[bi, half * NQS : (half + 1) * NQS, :] = (
            o.transpose(1, 0, 2).reshape(D, NQS).T)
    return out



# revision 18
# speedup vs baseline: 1.3615x; 1.3615x over previous
# Cross-attention kernel for Trainium2, 8 NeuronCores.
#
# Sharding: data-parallel over (batch, query-half): core = 2*b + half handles
# batch b, queries [half*1024, (half+1)*1024). No collectives.
#
# On-device layout is feature-major: activations live as [feature, token] in
# fp16 (8x less quantization noise than bf16 at identical PE/DVE rates). Both
# layernorms fold into the projections. Q/K use the *postscale* form
#   LN(x) @ W'.T = (x @ W' + [-S; bq] x [m; std]) * rstd,
# with W' = (W*g).T, S[j] = sum_d W'[d,j], bq = W @ b, std*rstd == 1 -- the
# input tensor is never rescaled in place, and the PSUM->SBUF evacuation copy
# becomes the rstd multiply. The finale keeps the prescale form so Gelu reads
# its PSUM accumulator directly.
#
# Attention runs keys-on-partitions; softmax denominators come from an
# all-ones column appended to V (row 64 of each O accumulator). The loop is
# c-outer (query-chunk outer, key-chunk inner) so one (P,c) pass needs only
# 2 PSUM banks of O-accumulator + 4 banks of double-buffered logits; the two
# spare banks let K/Q projection chunks for later head-pairs stream through
# the attention window under the ACT-bound exp stream. Max-subtraction is
# skipped: |logits/temp| < 3. 1/x everywhere is reciprocal_approx_fast (one
# custom-DVE op) instead of the 8-cycle/element iterative divide.
import os
import sys
import tempfile

# The neuron compile cache keys on the jax module hash, which does not cover
# the embedded Bass program — a stale NEFF can be silently reused. Use a
# fresh cache dir per process.
os.environ["NEURON_COMPILE_CACHE_URL"] = tempfile.mkdtemp(prefix="neff_cache_")
# The axon cassette (compile record/replay) fingerprints the module interface
# but not the embedded Bass program; salt it per process so edits always
# recompile instead of replaying a stale executable.
os.environ["AXON_CASSETTE_SALT"] = f"ca-{os.getpid()}-{os.urandom(4).hex()}"

for _p in ("/opt/trn_rl_repo",):
    if os.path.isdir(_p) and _p not in sys.path:
        sys.path.insert(0, _p)

import numpy as np
from contextlib import ExitStack

import concourse.bass as bass
import concourse.tile as tile
from concourse import bacc, mybir
from concourse.bass_utils import run_bass_kernel_spmd

F32 = mybir.dt.float32
F32R = mybir.dt.float32r
F16 = mybir.dt.float16
AF = mybir.ActivationFunctionType
ALU = mybir.AluOpType

B, NQ, NK, D = 4, 2048, 2048, 512
H, DH = 8, 64
NQS = NQ // 2  # queries per core
TEMP = float(np.sqrt(512.0))
LN_EPS = 1e-5
N_CORES = 8

_CACHE = {}
# Interface salt: the remote executable cache fingerprints the module
# interface but not the embedded program; bump per kernel edit to force
# recompilation.
SALT = "v11"
SALT_N = 11
DEBUG = os.environ.get("CA_DEBUG", "0") == "1"


def _build_program():
    nc = bacc.Bacc("TRN2", target_bir_lowering=False, debug=False)

    def din(name, shape, dt=F32):
        return nc.dram_tensor(f"{name}_{SALT}", shape, dt,
                              kind="ExternalInput").ap()

    qt_d = din("qt", [128, 4, NQS], F16)
    kt_d = din("kt", [128, 4, NK], F16)
    vt_d = din("vt", [128, 4, NK], F16)
    wq_d = din("wq", [128, 4, D], F16)
    wk_d = din("wk", [128, 4, D], F16)
    wv_d = din("wv", [128, 4, D], F16)
    wo_d = din("wo", [128, 4, D], F32R)
    aq_d = din("aq", [2, D], F32R)
    ak_d = din("ak", [2, D], F32R)
    ao_d = din("ao", [2, D], F32R)
    gb_d = din("gb", [128, 2, 4], F32)
    grow_d = din("grow", [1, D], F32R)
    salt_d = din("salt", [1, 8 + SALT_N], F32)
    out_d = nc.dram_tensor(f"out_{SALT}", [128, 4, NQS], F32,
                           kind="ExternalOutput").ap()
    if DEBUG:
        def dout(name, shape, dt):
            return nc.dram_tensor(f"{name}_{SALT}", shape, dt,
                                  kind="ExternalOutput").ap()
        dbg_kts = dout("dbg_kts", [128, 4, NK], F16)
        dbg_qts = dout("dbg_qts", [128, 4, NQS], F16)
        dbg_vaug = dout("dbg_vaug", [128, 16, 8, 65], F16)
        dbg_ots = dout("dbg_ots", [128, 4, NQS], F32)
        dbg_rr = dout("dbg_rr", [64, 512], F32)

    with tile.TileContext(nc) as tc, ExitStack() as top:
        persist = top.enter_context(tc.tile_pool(name="persist", bufs=1))
        qTs = persist.tile([128, 4, NQS], F16)        # projected q
        kTs = persist.tile([128, 4, NK], F16)         # projected k
        vaug = persist.tile([128, 16, 8, 65], F16)    # v natural + ones col
        oTs = persist.tile([128, 4, NQS], F32R)       # attention out
        qtin = persist.tile([128, 4, NQS], F16)
        ktin = persist.tile([128, 4, NK], F16)
        vtin = persist.tile([128, 4, NK], F16)
        wq_sb = persist.tile([128, 4, D], F16)
        wk_sb = persist.tile([128, 4, D], F16)
        wv_sb = persist.tile([128, 4, D], F16)
        wo_sb = persist.tile([128, 4, D], F32R)
        aq_sb = persist.tile([2, D], F32R)
        ak_sb = persist.tile([2, D], F32R)
        ao_sb = persist.tile([2, D], F32R)
        gb_sb = persist.tile([128, 2, 4], F32)
        grow_sb = persist.tile([1, D], F32R)
        auxq = persist.tile([2, NQS], F32R)           # [m; std] rows for Q
        auxk = persist.tile([2, NK], F32R)
        auxo = persist.tile([2, NQS], F32R)           # [m*r; 1] rows, finale
        rstdQ = persist.tile([128, 2, 512], F32)      # replicated 1/std per
        rstdK = persist.tile([128, 4, 512], F32)      # 512-token chunk
        onesh = persist.tile([128, 128], F16)
        onesr = persist.tile([128, 128], F32R)
        eps_t = persist.tile([128, 1], F32)

        nc.sync.dma_start(out=wk_sb, in_=wk_d)
        for n2 in range(4):
            ns = slice(512 * n2, 512 * n2 + 512)
            nc.sync.dma_start(out=ktin[:, :, ns], in_=kt_d[:, :, ns])
        nc.sync.dma_start(out=wv_sb, in_=wv_d)
        for n2 in range(4):
            ns = slice(512 * n2, 512 * n2 + 512)
            nc.sync.dma_start(out=vtin[:, :, ns], in_=vt_d[:, :, ns])
        nc.sync.dma_start(out=wq_sb, in_=wq_d)
        for n2 in range(2):
            ns = slice(512 * n2, 512 * n2 + 512)
            nc.sync.dma_start(out=qtin[:, :, ns], in_=qt_d[:, :, ns])
        nc.sync.dma_start(out=wo_sb, in_=wo_d)
        nc.sync.dma_start(out=aq_sb, in_=aq_d)
        nc.sync.dma_start(out=ak_sb, in_=ak_d)
        nc.sync.dma_start(out=ao_sb, in_=ao_d)
        nc.sync.dma_start(out=gb_sb, in_=gb_d)
        nc.sync.dma_start(out=grow_sb, in_=grow_d)
        nc.sync.dma_start(out=eps_t, in_=salt_d[0:1, 0:1].to_broadcast([128, 1]))
        nc.vector.memset(onesh, 1.0)
        # memset cannot target f32r; write through an f32 view (same bytes)
        nc.vector.memset(onesr.bitcast(F32), 1.0)
        nc.vector.memset(vaug[:, :, :, 64], 1.0)
        # row 0 is overwritten with m*r in the finale; row 1 stays all-ones
        nc.vector.memset(auxo.bitcast(F32), 1.0)

        pmm = top.enter_context(tc.tile_pool(name="pmm", bufs=1, space="PSUM"))
        work = top.enter_context(tc.tile_pool(name="work", bufs=1))

        def ln_stats(xin, n2, aux, rstd_store):
            """Column LN stats of token chunk n2 of xin [128, 4, *]; fills
            aux rows [m; std] and rstd_store[:, n2, :] (replicated)."""
            ns = slice(512 * n2, 512 * n2 + 512)
            ps_sum = pmm.tile([128, 512], F32, name="pmm", bufs=2)
            for kc in range(4):
                nc.tensor.matmul(ps_sum, onesh, xin[:, kc, ns],
                                 start=(kc == 0), stop=(kc == 3))
            ps_ssq = pmm.tile([128, 512], F32, name="pmm", bufs=2)
            for kc in range(4):
                sq = work.tile([128, 512], F16, name="sq", bufs=3)
                with nc.allow_low_precision("squares in fp16"):
                    nc.vector.tensor_mul(sq, xin[:, kc, ns], xin[:, kc, ns])
                nc.tensor.matmul(ps_ssq, onesh, sq,
                                 start=(kc == 0), stop=(kc == 3))
            m_b = work.tile([128, 512], F32, name="w32", bufs=6)
            nc.scalar.mul(m_b, ps_sum, 1.0 / 512.0)
            t2 = work.tile([128, 512], F32, name="w32", bufs=6)
            nc.vector.tensor_mul(t2, m_b, ps_sum)
            dv = work.tile([128, 512], F32, name="w32", bufs=6)
            nc.vector.tensor_sub(dv, ps_ssq, t2)
            std_b = work.tile([128, 512], F32, name="w32", bufs=6)
            nc.scalar.activation(std_b, dv, AF.Sqrt, bias=eps_t,
                                 scale=1.0 / 512.0)
            nc.vector.reciprocal_approx_fast(rstd_store[:, n2, :], std_b)
            # engine ops cannot move data across partitions; DMA the stat
            # rows into the K=2 aux operand instead.
            nc.sync.dma_start(out=aux[0:1, ns].bitcast(F32), in_=m_b[0:1, :])
            nc.sync.dma_start(out=aux[1:2, ns].bitcast(F32), in_=std_b[0:1, :])

        def proj_chunk(dst, xin, w_sb, a_sb, aux, rstd_store, jc, n2):
            """dst[:, jc, ns] = (sum_kc w'[:,kc,js].T @ x[:,kc,ns] + aux)*r."""
            ns = slice(512 * n2, 512 * n2 + 512)
            js = slice(128 * jc, 128 * jc + 128)
            pg = pmm.tile([128, 512], F32, name="pmm", bufs=2)
            for kc in range(4):
                nc.tensor.matmul(pg, w_sb[:, kc, js], xin[:, kc, ns],
                                 start=(kc == 0), stop=False)
            nc.tensor.matmul(pg, a_sb[:, js], aux[:, ns],
                             start=False, stop=True)
            with nc.allow_low_precision("fp16 activations"):
                nc.vector.tensor_mul(dst[:, jc, ns], pg, rstd_store[:, n2, :])

        # ---- Prefix: K stats + K proj jc0, V proj, Q stats + Q proj jc0 ----
        for n2 in range(4):
            ln_stats(ktin, n2, auxk, rstdK)
        for n2 in range(4):
            proj_chunk(kTs, ktin, wk_sb, ak_sb, auxk, rstdK, 0, n2)
        for t in range(16):
            ts = slice(128 * t, 128 * t + 128)
            pv = pmm.tile([128, 512], F32, name="pmm", bufs=2)
            for kc in range(4):
                nc.tensor.matmul(pv, vtin[:, kc, ts], wv_sb[:, kc, :],
                                 start=(kc == 0), stop=(kc == 3))
            with nc.allow_low_precision("fp16 activations"):
                nc.vector.tensor_copy(
                    vaug[:, t, :, 0:64],
                    pv.rearrange("p (h v) -> p h v", h=8))
        for n2 in range(2):
            ln_stats(qtin, n2, auxq, rstdQ)
        for n2 in range(2):
            proj_chunk(qTs, qtin, wq_sb, aq_sb, auxq, rstdQ, 0, n2)

        # Remaining projection chunks stream through the attention window
        # (the exp stream on ACT is the bottleneck there; PE has slack).
        tasks = {}

        def add_task(u, fn):
            tasks.setdefault(u, []).append(fn)

        def mk_proj(dst, xin, w_sb, a_sb, aux, rstd_store, jc, n2):
            return lambda: proj_chunk(dst, xin, w_sb, a_sb, aux, rstd_store,
                                      jc, n2)

        u0 = 2
        for jc in range(1, 4):
            base = u0 + 32 * (jc - 1)
            for n2 in range(4):
                add_task(base + 2 * n2, mk_proj(kTs, ktin, wk_sb, ak_sb,
                                                auxk, rstdK, jc, n2))
            for n2 in range(2):
                add_task(base + 8 + 2 * n2, mk_proj(qTs, qtin, wq_sb, aq_sb,
                                                    auxq, rstdQ, jc, n2))

        # ---- Attention: units (P, c, p); c-outer so one (P,c) pass holds
        # only 2 O-accumulator banks; psL double-buffered. ----
        at_psL = top.enter_context(tc.tile_pool(name="at_psL", bufs=1,
                                                space="PSUM"))
        at_po = top.enter_context(tc.tile_pool(name="at_po", bufs=1,
                                               space="PSUM"))
        at_sb = top.enter_context(tc.tile_pool(name="at_sb", bufs=1))

        units = [(P, c, p) for P in range(4) for c in range(2)
                 for p in range(16)]
        LAG = 2
        pend = {}   # unit idx -> (P, c, p, psO pair, ex tile)

        def emit_O(u):
            P, c, p, ps_o, ex = pend.pop(u)
            for hh in range(2):
                nc.tensor.matmul(ps_o[hh], vaug[:, p, 2 * P + hh, :],
                                 ex[:, 512 * hh: 512 * hh + 512],
                                 start=(p == 0), stop=(p == 15))
            if p == 15:
                cs = slice(512 * c, 512 * c + 512)
                for hh in range(2):
                    # recip_approx only works from SBUF at partition 0:
                    # evacuate the den row (aligned copy), DMA it to
                    # partition 0, then reciprocal + broadcast.
                    den = at_sb.tile([65, 512], F32, name="rr", bufs=2)
                    nc.vector.tensor_copy(den[64:65, :], ps_o[hh][64:65, :])
                    den0 = at_sb.tile([1, 512], F32, name="tl32", bufs=6)
                    nc.sync.dma_start(out=den0, in_=den[64:65, :])
                    rr0 = at_sb.tile([1, 512], F32, name="tl32", bufs=6)
                    nc.vector.reciprocal_approx_fast(rr0, den0)
                    rrb = at_sb.tile([64, 512], F32, name="tl32", bufs=6)
                    nc.gpsimd.partition_broadcast(rrb, rr0)
                    ost = at_sb.tile([64, 512], F32, name="tl32", bufs=6)
                    nc.vector.tensor_mul(ost, ps_o[hh][0:64, :], rrb)
                    rb = 64 * hh
                    nc.sync.dma_start(
                        out=oTs[rb:rb + 64, P, cs].bitcast(F32), in_=ost)
                    if DEBUG and P == 0 and c == 0 and hh == 0:
                        nc.sync.dma_start(out=dbg_rr, in_=rrb)

        ps_o_cur = None
        for u, (P, c, p) in enumerate(units):
            if p == 0:
                ps_o_cur = [at_po.tile([65, 512], F32, name=f"po{hh}",
                                       bufs=1) for hh in range(2)]
            psL = at_psL.tile([128, 1024], F32, name="psL", bufs=2)
            ks = slice(128 * p, 128 * p + 128)
            cs = slice(512 * c, 512 * c + 512)
            for hh in range(2):
                rb = 64 * hh
                nc.tensor.matmul(psL[:, 512 * hh: 512 * hh + 512],
                                 kTs[rb:rb + 64, P, ks],
                                 qTs[rb:rb + 64, P, cs],
                                 start=True, stop=True)
            ex = at_sb.tile([128, 1024], F16, name="ex", bufs=LAG + 1)
            nc.scalar.activation(ex, psL, AF.Exp, scale=1.0 / TEMP)
            pend[u] = (P, c, p, ps_o_cur, ex)
            if u >= LAG:
                emit_O(u - LAG)
            for fn in tasks.pop(u, ()):
                fn()
        for u in range(len(units) - LAG, len(units)):
            emit_O(u)

        if DEBUG:
            nc.sync.dma_start(out=dbg_kts, in_=kTs)
            nc.sync.dma_start(out=dbg_qts, in_=qTs)
            nc.sync.dma_start(out=dbg_vaug, in_=vaug)
            nc.sync.dma_start(out=dbg_ots, in_=oTs.bitcast(F32))

        # ---- Finale: LN fold (prescale form) + Wo + gelu + residual ----
        for n2 in range(2):
            ns = slice(512 * n2, 512 * n2 + 512)
            ps_sum = pmm.tile([128, 512], F32, name="pmm", bufs=2)
            for jc in range(4):
                nc.tensor.matmul(ps_sum, onesr, oTs[:, jc, ns],
                                 start=(jc == 0), stop=(jc == 3))
            ps_ssq = pmm.tile([128, 512], F32, name="pmm", bufs=2)
            for jc in range(4):
                sqo = work.tile([128, 512], F32R, name="w32", bufs=6)
                with nc.allow_low_precision("f32r keeps fp32 storage"):
                    nc.vector.tensor_mul(sqo, oTs[:, jc, ns], oTs[:, jc, ns])
                nc.tensor.matmul(ps_ssq, onesr, sqo,
                                 start=(jc == 0), stop=(jc == 3))
            m_b = work.tile([128, 512], F32, name="w32", bufs=6)
            nc.scalar.mul(m_b, ps_sum, 1.0 / 512.0)
            t2 = work.tile([128, 512], F32, name="w32", bufs=6)
            nc.vector.tensor_mul(t2, m_b, ps_sum)
            dv = work.tile([128, 512], F32, name="w32", bufs=6)
            nc.vector.tensor_sub(dv, ps_ssq, t2)
            std_b = work.tile([128, 512], F32, name="w32", bufs=6)
            nc.scalar.activation(std_b, dv, AF.Sqrt, bias=eps_t,
                                 scale=1.0 / 512.0)
            r_b = work.tile([128, 512], F32, name="w32", bufs=6)
            nc.vector.reciprocal_approx_fast(r_b, std_b)
            with nc.allow_low_precision("f32r keeps fp32 storage"):
                nc.vector.tensor_mul(auxo[0:1, ns], m_b[0:1, :], r_b[0:1, :])
                for jc in range(4):
                    nc.vector.tensor_mul(oTs[:, jc, ns], oTs[:, jc, ns], r_b)

        for jc in range(4):
            js = slice(128 * jc, 128 * jc + 128)
            for n2 in range(2):
                ns = slice(512 * n2, 512 * n2 + 512)
                pg = pmm.tile([128, 512], F32, name="pmm", bufs=2)
                for kc in range(4):
                    nc.tensor.matmul(pg, wo_sb[:, kc, js], oTs[:, kc, ns],
                                     start=(kc == 0), stop=False)
                nc.tensor.matmul(pg, ao_sb[:, js], auxo[:, ns],
                                 start=False, stop=True)
                pbm = pmm.tile([128, 512], F32, name="pmm", bufs=2)
                nc.tensor.matmul(pbm, grow_sb[0:1, js], auxo[0:1, ns],
                                 start=True, stop=True)
                gl = work.tile([128, 512], F32, name="w32", bufs=6)
                nc.scalar.activation(gl, pg, AF.Gelu)
                u2 = work.tile([128, 512], F32, name="w32", bufs=6)
                # u2 = oTs_scaled*g - m*r*g   (oTs already prescaled by r)
                nc.vector.scalar_tensor_tensor(
                    u2, oTs[:, jc, ns], gb_sb[:, 0, jc:jc + 1], pbm,
                    op0=ALU.mult, op1=ALU.subtract)
                of = work.tile([128, 512], F32, name="w32", bufs=6)
                nc.vector.scalar_tensor_tensor(
                    of, u2, gb_sb[:, 1, jc:jc + 1], gl,
                    op0=ALU.add, op1=ALU.add)
                nc.sync.dma_start(out=out_d[:, jc, ns], in_=of)

    nc.compile()
    return nc


def _chunk_fm(x):
    """[512, N] feature-major -> [128, 4, N] (partition, chunk, col)."""
    n = x.shape[1]
    return np.ascontiguousarray(x.reshape(4, 128, n).transpose(1, 0, 2))


def _prep_inputs(Q, K, V, Wq, Wk, Wv, Wo, g, b, go, bo):
    WqT = np.ascontiguousarray((Wq * g[None, :]).T)
    WkT = np.ascontiguousarray((Wk * g[None, :]).T)
    WvT = np.ascontiguousarray(Wv.T)
    WoT = np.ascontiguousarray((Wo * go[None, :]).T)
    f16 = np.float16
    shared = {
        f"wq_{SALT}": _chunk_fm(WqT).astype(f16),
        f"wk_{SALT}": _chunk_fm(WkT).astype(f16),
        f"wv_{SALT}": _chunk_fm(WvT).astype(f16),
        f"wo_{SALT}": _chunk_fm(WoT),
        f"aq_{SALT}": np.ascontiguousarray(np.stack([-WqT.sum(0), Wq @ b])),
        f"ak_{SALT}": np.ascontiguousarray(np.stack([-WkT.sum(0), Wk @ b])),
        f"ao_{SALT}": np.ascontiguousarray(np.stack([-WoT.sum(0), Wo @ bo])),
        f"gb_{SALT}": np.ascontiguousarray(
            np.stack([go.reshape(4, 128).T, bo.reshape(4, 128).T], axis=1)),
        f"grow_{SALT}": np.ascontiguousarray(go[None, :]),
    }
    in_maps = []
    for core in range(N_CORES):
        bi, half = core // 2, core % 2
        qs = slice(half * NQS, (half + 1) * NQS)
        m = dict(shared)
        m[f"salt_{SALT}"] = np.full((1, 8 + SALT_N), LN_EPS, np.float32)
        m[f"qt_{SALT}"] = _chunk_fm(np.ascontiguousarray(Q[bi, qs, :].T)).astype(f16)
        m[f"kt_{SALT}"] = _chunk_fm(np.ascontiguousarray(K[bi].T)).astype(f16)
        m[f"vt_{SALT}"] = _chunk_fm(np.ascontiguousarray(V[bi].T)).astype(f16)
        in_maps.append(m)
    return in_maps


def kernel(Q, K, V, Wq, Wk, Wv, Wo, ln_qk_g, ln_qk_b, ln_o_g, ln_o_b,
           _trace=False):
    args = [np.asarray(a, dtype=np.float32) for a in
            (Q, K, V, Wq, Wk, Wv, Wo, ln_qk_g, ln_qk_b, ln_o_g, ln_o_b)]
    if "nc" not in _CACHE:
        _CACHE["nc"] = _build_program()
    nc = _CACHE["nc"]
    in_maps = _prep_inputs(*args)
    res = run_bass_kernel_spmd(nc, in_maps, core_ids=list(range(N_CORES)),
                               trace=_trace)
    _CACHE["last_results"] = res
    out = np.empty((B, NQ, D), dtype=np.float32)
    for core in range(N_CORES):
        bi, half = core // 2, core % 2
        o = res.results[core][f"out_{SALT}"]  # [128, 4, NQS]
        out[bi, half * NQS : (half + 1) * NQS, :] = (
            o.transpose(1, 0, 2).reshape(D, NQS).T)
    return out


# revision 27
# speedup vs baseline: 1.6653x; 1.2231x over previous
# Cross-attention kernel for Trainium2, 8 NeuronCores.
#
# Sharding: data-parallel over (batch, query-half): core = 2*b + half handles
# batch b, queries [half*1024, (half+1)*1024). No collectives.
#
# On-device layout is feature-major: activations live as [feature, token] in
# fp16 (8x less quantization noise than bf16 at identical PE/DVE rates). Both
# layernorms fold into the projections. Q/K use the *postscale* form
#   LN(x) @ W'.T = (x @ W' + [-S; bq] x [m; std]) * rstd,
# with W' = (W*g).T, S[j] = sum_d W'[d,j], bq = W @ b, std*rstd == 1 -- the
# input tensor is never rescaled in place, and the PSUM->SBUF evacuation copy
# becomes the rstd multiply. The finale keeps the prescale form so Gelu reads
# its PSUM accumulator directly.
#
# Attention runs keys-on-partitions; softmax denominators come from an
# all-ones column appended to V (row 64 of each O accumulator). The loop is
# c-outer (query-chunk outer, key-chunk inner) so one (P,c) pass needs only
# 2 PSUM banks of O-accumulator + 4 banks of double-buffered logits; the two
# spare banks let K/Q projection chunks for later head-pairs stream through
# the attention window under the ACT-bound exp stream. Max-subtraction is
# skipped: |logits/temp| < 3. 1/x everywhere is reciprocal_approx_fast (one
# custom-DVE op) instead of the 8-cycle/element iterative divide.
import os
import sys
import tempfile

# The neuron compile cache keys on the jax module hash, which does not cover
# the embedded Bass program — a stale NEFF can be silently reused. Use a
# fresh cache dir per process.
os.environ["NEURON_COMPILE_CACHE_URL"] = tempfile.mkdtemp(prefix="neff_cache_")
# The axon cassette (compile record/replay) fingerprints the module interface
# but not the embedded Bass program; salt it per process so edits always
# recompile instead of replaying a stale executable.
os.environ["AXON_CASSETTE_SALT"] = f"ca-{os.getpid()}-{os.urandom(4).hex()}"

for _p in ("/opt/trn_rl_repo",):
    if os.path.isdir(_p) and _p not in sys.path:
        sys.path.insert(0, _p)

import numpy as np
from contextlib import ExitStack

import concourse.bass as bass
import concourse.tile as tile
from concourse import bacc, mybir
from concourse.bass_utils import run_bass_kernel_spmd

F32 = mybir.dt.float32
F32R = mybir.dt.float32r
F16 = mybir.dt.float16
AF = mybir.ActivationFunctionType
ALU = mybir.AluOpType

B, NQ, NK, D = 4, 2048, 2048, 512
H, DH = 8, 64
NQS = NQ // 2  # queries per core
TEMP = float(np.sqrt(512.0))
LN_EPS = 1e-5
N_CORES = 8

_CACHE = {}
# Interface salt: the remote executable cache fingerprints the module
# interface but not the embedded program; bump per kernel edit to force
# recompilation.
SALT = "v12"
SALT_N = 12
DEBUG = os.environ.get("CA_DEBUG", "0") == "1"


def _build_program():
    nc = bacc.Bacc("TRN2", target_bir_lowering=False, debug=False)

    def din(name, shape, dt=F32):
        return nc.dram_tensor(f"{name}_{SALT}", shape, dt,
                              kind="ExternalInput").ap()

    qt_d = din("qt", [128, 4, NQS], F16)
    kt_d = din("kt", [128, 4, NK], F16)
    vt_d = din("vt", [128, 4, NK], F16)
    wq_d = din("wq", [128, 4, D], F16)
    wk_d = din("wk", [128, 4, D], F16)
    wv_d = din("wv", [128, 4, D], F16)
    wo_d = din("wo", [128, 4, D], F32R)
    aq_d = din("aq", [2, D], F32R)
    ak_d = din("ak", [2, D], F32R)
    ao_d = din("ao", [2, D], F32R)
    gb_d = din("gb", [128, 2, 4], F32)
    grow_d = din("grow", [1, D], F32R)
    salt_d = din("salt", [1, 8 + SALT_N], F32)
    out_d = nc.dram_tensor(f"out_{SALT}", [128, 4, NQS], F32,
                           kind="ExternalOutput").ap()
    if DEBUG:
        def dout(name, shape, dt):
            return nc.dram_tensor(f"{name}_{SALT}", shape, dt,
                                  kind="ExternalOutput").ap()
        dbg_kts = dout("dbg_kts", [128, 4, NK], F16)
        dbg_qts = dout("dbg_qts", [128, 4, NQS], F16)
        dbg_vaug = dout("dbg_vaug", [128, 16, 8, 65], F16)
        dbg_ots = dout("dbg_ots", [128, 4, NQS], F32)
        dbg_rr = dout("dbg_rr", [64, 512], F32)

    with tile.TileContext(nc) as tc, ExitStack() as top:
        persist = top.enter_context(tc.tile_pool(name="persist", bufs=1))
        qTs = persist.tile([128, 4, NQS], F16)        # projected q
        kTs = persist.tile([128, 4, NK], F16)         # projected k
        vaug = persist.tile([128, 16, 8, 65], F16)    # v natural + ones col
        oTs = persist.tile([128, 4, NQS], F32R)       # attention out
        qtin = persist.tile([128, 4, NQS], F16)
        ktin = persist.tile([128, 4, NK], F16)
        vtin = persist.tile([128, 4, NK], F16)
        wq_sb = persist.tile([128, 4, D], F16)
        wk_sb = persist.tile([128, 4, D], F16)
        wv_sb = persist.tile([128, 4, D], F16)
        wo_sb = persist.tile([128, 4, D], F32R)
        aq_sb = persist.tile([2, D], F32R)
        ak_sb = persist.tile([2, D], F32R)
        ao_sb = persist.tile([2, D], F32R)
        gb_sb = persist.tile([128, 2, 4], F32)
        grow_sb = persist.tile([1, D], F32R)
        auxq = persist.tile([2, NQS], F32R)           # [m; std] rows for Q
        auxk = persist.tile([2, NK], F32R)
        auxo = persist.tile([2, NQS], F32R)           # [m*r; 1] rows, finale
        rstdQ = persist.tile([128, 2, 512], F32)      # replicated 1/std per
        rstdK = persist.tile([128, 4, 512], F32)      # 512-token chunk
        onesh = persist.tile([128, 128], F16)
        onesr = persist.tile([128, 128], F32R)
        eps_t = persist.tile([128, 1], F32)

        # Small latency-critical transfers first: the stats chains need
        # eps/weights immediately; anything queued behind the 5MB of input
        # DMAs stalls the whole prefix.
        nc.sync.dma_start(out=eps_t, in_=salt_d[0:1, 0:1].to_broadcast([128, 1]))
        nc.sync.dma_start(out=wk_sb, in_=wk_d)
        nc.sync.dma_start(out=wq_sb, in_=wq_d)
        nc.sync.dma_start(out=aq_sb, in_=aq_d)
        nc.sync.dma_start(out=ak_sb, in_=ak_d)
        for n2 in range(4):
            ns = slice(512 * n2, 512 * n2 + 512)
            nc.sync.dma_start(out=ktin[:, :, ns], in_=kt_d[:, :, ns])
        for n2 in range(2):
            ns = slice(512 * n2, 512 * n2 + 512)
            nc.sync.dma_start(out=qtin[:, :, ns], in_=qt_d[:, :, ns])
        nc.sync.dma_start(out=wv_sb, in_=wv_d)
        for n2 in range(4):
            ns = slice(512 * n2, 512 * n2 + 512)
            nc.sync.dma_start(out=vtin[:, :, ns], in_=vt_d[:, :, ns])
        nc.sync.dma_start(out=wo_sb, in_=wo_d)
        nc.sync.dma_start(out=ao_sb, in_=ao_d)
        nc.sync.dma_start(out=gb_sb, in_=gb_d)
        nc.sync.dma_start(out=grow_sb, in_=grow_d)
        nc.vector.memset(onesh, 1.0)
        # memset cannot target f32r; write through an f32 view (same bytes)
        nc.vector.memset(onesr.bitcast(F32), 1.0)
        nc.vector.memset(vaug[:, :, :, 64], 1.0)
        # row 0 is overwritten with m*r in the finale; row 1 stays all-ones
        nc.vector.memset(auxo.bitcast(F32), 1.0)

        pmm = top.enter_context(tc.tile_pool(name="pmm", bufs=1, space="PSUM"))
        work = top.enter_context(tc.tile_pool(name="work", bufs=1))
        # Prefix-only stats accumulators: separate banks so the stats chain
        # of chunk n+1 never waits on a projection accumulator and vice
        # versa. Closed before the attention pools open.
        pre_ctx = tc.tile_pool(name="pre", bufs=1, space="PSUM")
        pre = pre_ctx.__enter__()

        def ln_stats(xin, n2, aux, rstd_store, pool):
            """Column LN stats of token chunk n2 of xin [128, 4, *]; fills
            aux rows [m; std] and rstd_store[:, n2, :] (replicated)."""
            ns = slice(512 * n2, 512 * n2 + 512)
            ps_sum = pool.tile([128, 512], F32, name="st", bufs=4)
            for kc in range(4):
                nc.tensor.matmul(ps_sum, onesh, xin[:, kc, ns],
                                 start=(kc == 0), stop=(kc == 3))
            ps_ssq = pool.tile([128, 512], F32, name="st", bufs=4)
            for kc in range(4):
                sq = work.tile([128, 512], F16, name="sq", bufs=3)
                with nc.allow_low_precision("squares in fp16"):
                    nc.vector.tensor_mul(sq, xin[:, kc, ns], xin[:, kc, ns])
                nc.tensor.matmul(ps_ssq, onesh, sq,
                                 start=(kc == 0), stop=(kc == 3))
            m_b = work.tile([128, 512], F32, name="w32", bufs=6)
            nc.scalar.mul(m_b, ps_sum, 1.0 / 512.0)
            t2 = work.tile([128, 512], F32, name="w32", bufs=6)
            nc.vector.tensor_mul(t2, m_b, ps_sum)
            dv = work.tile([128, 512], F32, name="w32", bufs=6)
            nc.vector.tensor_sub(dv, ps_ssq, t2)
            std_b = work.tile([128, 512], F32, name="w32", bufs=6)
            nc.scalar.activation(std_b, dv, AF.Sqrt, bias=eps_t,
                                 scale=1.0 / 512.0)
            nc.vector.reciprocal_approx_fast(rstd_store[:, n2, :], std_b)
            # engine ops cannot move data across partitions; DMA the stat
            # rows into the K=2 aux operand instead.
            nc.sync.dma_start(out=aux[0:1, ns].bitcast(F32), in_=m_b[0:1, :])
            nc.sync.dma_start(out=aux[1:2, ns].bitcast(F32), in_=std_b[0:1, :])

        def proj_chunk(dst, xin, w_sb, a_sb, aux, rstd_store, jc, n2):
            """dst[:, jc, ns] = (sum_kc w'[:,kc,js].T @ x[:,kc,ns] + aux)*r."""
            ns = slice(512 * n2, 512 * n2 + 512)
            js = slice(128 * jc, 128 * jc + 128)
            pg = pmm.tile([128, 512], F32, name="pmm", bufs=2)
            for kc in range(4):
                nc.tensor.matmul(pg, w_sb[:, kc, js], xin[:, kc, ns],
                                 start=(kc == 0), stop=False)
            nc.tensor.matmul(pg, a_sb[:, js], aux[:, ns],
                             start=False, stop=True)
            with nc.allow_low_precision("fp16 activations"):
                nc.vector.tensor_mul(dst[:, jc, ns], pg, rstd_store[:, n2, :])

        # ---- Prefix: K stats + K proj jc0, V proj, Q stats + Q proj jc0 ----
        for n2 in range(4):
            ln_stats(ktin, n2, auxk, rstdK, pre)
        for n2 in range(4):
            proj_chunk(kTs, ktin, wk_sb, ak_sb, auxk, rstdK, 0, n2)
        for t in range(16):
            ts = slice(128 * t, 128 * t + 128)
            pv = pmm.tile([128, 512], F32, name="pmm", bufs=2)
            for kc in range(4):
                nc.tensor.matmul(pv, vtin[:, kc, ts], wv_sb[:, kc, :],
                                 start=(kc == 0), stop=(kc == 3))
            with nc.allow_low_precision("fp16 activations"):
                nc.vector.tensor_copy(
                    vaug[:, t, :, 0:64],
                    pv.rearrange("p (h v) -> p h v", h=8))
        for n2 in range(2):
            ln_stats(qtin, n2, auxq, rstdQ, pre)
        for n2 in range(2):
            proj_chunk(qTs, qtin, wq_sb, aq_sb, auxq, rstdQ, 0, n2)
        pre_ctx.__exit__(None, None, None)  # release stats banks for psL

        # Remaining projection chunks stream through the attention window
        # (the exp stream on ACT is the bottleneck there; PE has slack).
        tasks = {}

        def add_task(u, fn):
            tasks.setdefault(u, []).append(fn)

        def mk_proj(dst, xin, w_sb, a_sb, aux, rstd_store, jc, n2):
            return lambda: proj_chunk(dst, xin, w_sb, a_sb, aux, rstd_store,
                                      jc, n2)

        u0 = 2
        for jc in range(1, 4):
            base = u0 + 32 * (jc - 1)
            for n2 in range(4):
                add_task(base + 4 * n2, mk_proj(kTs, ktin, wk_sb, ak_sb,
                                                auxk, rstdK, jc, n2))
            for n2 in range(2):
                add_task(base + 16 + 4 * n2, mk_proj(qTs, qtin, wq_sb, aq_sb,
                                                     auxq, rstdQ, jc, n2))

        # ---- Attention: units (P, c, p); c-outer so one (P,c) pass holds
        # only 2 O-accumulator banks; psL double-buffered. ----
        at_psL = top.enter_context(tc.tile_pool(name="at_psL", bufs=1,
                                                space="PSUM"))
        at_po = top.enter_context(tc.tile_pool(name="at_po", bufs=1,
                                               space="PSUM"))
        at_sb = top.enter_context(tc.tile_pool(name="at_sb", bufs=1))

        units = [(P, c, p) for P in range(4) for c in range(2)
                 for p in range(16)]
        LAG = 4
        pend = {}   # unit idx -> (P, c, p, psO pair, ex tile)

        def emit_O(u):
            P, c, p, ps_o, ex = pend.pop(u)
            for hh in range(2):
                nc.tensor.matmul(ps_o[hh], vaug[:, p, 2 * P + hh, :],
                                 ex[:, 512 * hh: 512 * hh + 512],
                                 start=(p == 0), stop=(p == 15))
            if p == 15:
                cs = slice(512 * c, 512 * c + 512)
                for hh in range(2):
                    # Evacuate the accumulator bank with two quick aligned
                    # DVE copies so the next (P,c) group's first O-matmul
                    # isn't blocked behind the whole normalize chain.
                    den = at_sb.tile([65, 512], F32, name="rr", bufs=2)
                    nc.vector.tensor_copy(den[64:65, :], ps_o[hh][64:65, :])
                    nst = at_sb.tile([64, 512], F32, name="nst", bufs=2)
                    nc.vector.tensor_copy(nst, ps_o[hh][0:64, :])
                    # recip_approx only works from SBUF at partition 0: DMA
                    # the den row down, then reciprocal + broadcast.
                    den0 = at_sb.tile([1, 512], F32, name="tl32", bufs=6)
                    nc.sync.dma_start(out=den0, in_=den[64:65, :])
                    rr0 = at_sb.tile([1, 512], F32, name="tl32", bufs=6)
                    nc.vector.reciprocal_approx_fast(rr0, den0)
                    rrb = at_sb.tile([64, 512], F32, name="tl32", bufs=6)
                    nc.gpsimd.partition_broadcast(rrb, rr0)
                    rb = 64 * hh
                    if hh == 0:
                        # rows align with oTs: write the normalized block
                        # in place, no DMA hop.
                        with nc.allow_low_precision("f32r storage"):
                            nc.vector.tensor_mul(oTs[0:64, P, cs], nst, rrb)
                    else:
                        ost = at_sb.tile([64, 512], F32, name="tl32", bufs=6)
                        nc.vector.tensor_mul(ost, nst, rrb)
                        nc.sync.dma_start(
                            out=oTs[rb:rb + 64, P, cs].bitcast(F32), in_=ost)
                    if DEBUG and P == 0 and c == 0 and hh == 0:
                        nc.sync.dma_start(out=dbg_rr, in_=rrb)

        # ---- Finale: LN fold (prescale form) + Wo + gelu + residual.
        # The n2=0 half of the stats (matmuls + DVE chain, no Sqrt — a Sqrt
        # would thrash the ACT table set mid-exp-stream) is emitted as a
        # task inside the attention window once all c=0 groups are done. ----
        fin = {}

        def fin_early(n2):
            ns = slice(512 * n2, 512 * n2 + 512)
            ps_sum = pmm.tile([128, 512], F32, name="pmm", bufs=2)
            for jc in range(4):
                nc.tensor.matmul(ps_sum, onesr, oTs[:, jc, ns],
                                 start=(jc == 0), stop=(jc == 3))
            ps_ssq = pmm.tile([128, 512], F32, name="pmm", bufs=2)
            for jc in range(4):
                sqo = work.tile([128, 512], F32R, name="w32", bufs=6)
                with nc.allow_low_precision("f32r keeps fp32 storage"):
                    nc.vector.tensor_mul(sqo, oTs[:, jc, ns], oTs[:, jc, ns])
                nc.tensor.matmul(ps_ssq, onesr, sqo,
                                 start=(jc == 0), stop=(jc == 3))
            m_b = work.tile([128, 512], F32, name="w32", bufs=6)
            nc.scalar.mul(m_b, ps_sum, 1.0 / 512.0)
            t2 = work.tile([128, 512], F32, name="w32", bufs=6)
            nc.vector.tensor_mul(t2, m_b, ps_sum)
            dv = work.tile([128, 512], F32, name="w32", bufs=6)
            nc.vector.tensor_sub(dv, ps_ssq, t2)
            fin[n2] = (m_b, dv)

        def fin_late(n2):
            ns = slice(512 * n2, 512 * n2 + 512)
            m_b, dv = fin[n2]
            std_b = work.tile([128, 512], F32, name="w32", bufs=6)
            nc.scalar.activation(std_b, dv, AF.Sqrt, bias=eps_t,
                                 scale=1.0 / 512.0)
            r_b = work.tile([128, 512], F32, name="w32", bufs=6)
            nc.vector.reciprocal_approx_fast(r_b, std_b)
            with nc.allow_low_precision("f32r keeps fp32 storage"):
                nc.vector.tensor_mul(auxo[0:1, ns], m_b[0:1, :], r_b[0:1, :])
                for jc in range(4):
                    nc.vector.tensor_mul(oTs[:, jc, ns], oTs[:, jc, ns], r_b)

        def fin_wo(jc, n2):
            js = slice(128 * jc, 128 * jc + 128)
            ns = slice(512 * n2, 512 * n2 + 512)
            pg = pmm.tile([128, 512], F32, name="pmm", bufs=2)
            for kc in range(4):
                nc.tensor.matmul(pg, wo_sb[:, kc, js], oTs[:, kc, ns],
                                 start=(kc == 0), stop=False)
            nc.tensor.matmul(pg, ao_sb[:, js], auxo[:, ns],
                             start=False, stop=True)
            pbm = pmm.tile([128, 512], F32, name="pmm", bufs=2)
            nc.tensor.matmul(pbm, grow_sb[0:1, js], auxo[0:1, ns],
                             start=True, stop=True)
            gl = work.tile([128, 512], F32, name="w32", bufs=6)
            nc.scalar.activation(gl, pg, AF.Gelu)
            u2 = work.tile([128, 512], F32, name="w32", bufs=6)
            # u2 = oTs_scaled*g - m*r*g   (oTs already prescaled by r)
            nc.vector.scalar_tensor_tensor(
                u2, oTs[:, jc, ns], gb_sb[:, 0, jc:jc + 1], pbm,
                op0=ALU.mult, op1=ALU.subtract)
            of = work.tile([128, 512], F32, name="w32", bufs=6)
            nc.vector.scalar_tensor_tensor(
                of, u2, gb_sb[:, 1, jc:jc + 1], gl,
                op0=ALU.add, op1=ALU.add)
            nc.sync.dma_start(out=out_d[:, jc, ns], in_=of)

        add_task(120, lambda: fin_early(0))

        ps_o_cur = None
        for u, (P, c, p) in enumerate(units):
            if p == 0:
                ps_o_cur = [at_po.tile([65, 512], F32, name=f"po{hh}",
                                       bufs=1) for hh in range(2)]
            psL = at_psL.tile([128, 1024], F32, name="psL", bufs=2)
            ks = slice(128 * p, 128 * p + 128)
            cs = slice(512 * c, 512 * c + 512)
            for hh in range(2):
                rb = 64 * hh
                nc.tensor.matmul(psL[:, 512 * hh: 512 * hh + 512],
                                 kTs[rb:rb + 64, P, ks],
                                 qTs[rb:rb + 64, P, cs],
                                 start=True, stop=True)
            ex = at_sb.tile([128, 1024], F16, name="ex", bufs=LAG + 1)
            nc.scalar.activation(ex, psL, AF.Exp, scale=1.0 / TEMP)
            pend[u] = (P, c, p, ps_o_cur, ex)
            if u >= LAG:
                emit_O(u - LAG)
            for fn in tasks.pop(u, ()):
                fn()
        for u in range(len(units) - LAG, len(units)):
            emit_O(u)

        if DEBUG:
            nc.sync.dma_start(out=dbg_kts, in_=kTs)
            nc.sync.dma_start(out=dbg_qts, in_=qTs)
            nc.sync.dma_start(out=dbg_vaug, in_=vaug)
            nc.sync.dma_start(out=dbg_ots, in_=oTs.bitcast(F32))

        fin_late(0)
        fin_early(1)
        for jc in range(4):
            fin_wo(jc, 0)
        fin_late(1)
        for jc in range(4):
            fin_wo(jc, 1)

    nc.compile()
    return nc


def _chunk_fm(x):
    """[512, N] feature-major -> [128, 4, N] (partition, chunk, col)."""
    n = x.shape[1]
    return np.ascontiguousarray(x.reshape(4, 128, n).transpose(1, 0, 2))


def _prep_inputs(Q, K, V, Wq, Wk, Wv, Wo, g, b, go, bo):
    WqT = np.ascontiguousarray((Wq * g[None, :]).T)
    WkT = np.ascontiguousarray((Wk * g[None, :]).T)
    WvT = np.ascontiguousarray(Wv.T)
    WoT = np.ascontiguousarray((Wo * go[None, :]).T)
    f16 = np.float16
    shared = {
        f"wq_{SALT}": _chunk_fm(WqT).astype(f16),
        f"wk_{SALT}": _chunk_fm(WkT).astype(f16),
        f"wv_{SALT}": _chunk_fm(WvT).astype(f16),
        f"wo_{SALT}": _chunk_fm(WoT),
        f"aq_{SALT}": np.ascontiguousarray(np.stack([-WqT.sum(0), Wq @ b])),
        f"ak_{SALT}": np.ascontiguousarray(np.stack([-WkT.sum(0), Wk @ b])),
        f"ao_{SALT}": np.ascontiguousarray(np.stack([-WoT.sum(0), Wo @ bo])),
        f"gb_{SALT}": np.ascontiguousarray(
            np.stack([go.reshape(4, 128).T, bo.reshape(4, 128).T], axis=1)),
        f"grow_{SALT}": np.ascontiguousarray(go[None, :]),
    }
    in_maps = []
    for core in range(N_CORES):
        bi, half = core // 2, core % 2
        qs = slice(half * NQS, (half + 1) * NQS)
        m = dict(shared)
        m[f"salt_{SALT}"] = np.full((1, 8 + SALT_N), LN_EPS, np.float32)
        m[f"qt_{SALT}"] = _chunk_fm(np.ascontiguousarray(Q[bi, qs, :].T)).astype(f16)
        m[f"kt_{SALT}"] = _chunk_fm(np.ascontiguousarray(K[bi].T)).astype(f16)
        m[f"vt_{SALT}"] = _chunk_fm(np.ascontiguousarray(V[bi].T)).astype(f16)
        in_maps.append(m)
    return in_maps


def kernel(Q, K, V, Wq, Wk, Wv, Wo, ln_qk_g, ln_qk_b, ln_o_g, ln_o_b,
           _trace=False):
    args = [np.asarray(a, dtype=np.float32) for a in
            (Q, K, V, Wq, Wk, Wv, Wo, ln_qk_g, ln_qk_b, ln_o_g, ln_o_b)]
    if "nc" not in _CACHE:
        _CACHE["nc"] = _build_program()
    nc = _CACHE["nc"]
    in_maps = _prep_inputs(*args)
    res = run_bass_kernel_spmd(nc, in_maps, core_ids=list(range(N_CORES)),
                               trace=_trace)
    _CACHE["last_results"] = res
    out = np.empty((B, NQ, D), dtype=np.float32)
    for core in range(N_CORES):
        bi, half = core // 2, core % 2
        o = res.results[core][f"out_{SALT}"]  # [128, 4, NQS]
        out[bi, half * NQS : (half + 1) * NQS, :] = (
            o.transpose(1, 0, 2).reshape(D, NQS).T)
    return out


# revision 29
# speedup vs baseline: 1.7034x; 1.0229x over previous
# Cross-attention kernel for Trainium2, 8 NeuronCores.
#
# Sharding: data-parallel over (batch, query-half): core = 2*b + half handles
# batch b, queries [half*1024, (half+1)*1024). No collectives.
#
# On-device layout is feature-major: activations live as [feature, token] in
# fp16 (8x less quantization noise than bf16 at identical PE/DVE rates). Both
# layernorms fold into the projections. Q/K use the *postscale* form
#   LN(x) @ W'.T = (x @ W' + [-S; bq] x [m; std]) * rstd,
# with W' = (W*g).T, S[j] = sum_d W'[d,j], bq = W @ b, std*rstd == 1 -- the
# input tensor is never rescaled in place, and the PSUM->SBUF evacuation copy
# becomes the rstd multiply. The finale keeps the prescale form so Gelu reads
# its PSUM accumulator directly.
#
# Attention runs keys-on-partitions; softmax denominators come from an
# all-ones column appended to V (row 64 of each O accumulator). The loop is
# c-outer (query-chunk outer, key-chunk inner) so one (P,c) pass needs only
# 2 PSUM banks of O-accumulator + 4 banks of double-buffered logits; the two
# spare banks let K/Q projection chunks for later head-pairs stream through
# the attention window under the ACT-bound exp stream. Max-subtraction is
# skipped: |logits/temp| < 3. 1/x everywhere is reciprocal_approx_fast (one
# custom-DVE op) instead of the 8-cycle/element iterative divide.
import os
import sys
import tempfile

# The neuron compile cache keys on the jax module hash, which does not cover
# the embedded Bass program — a stale NEFF can be silently reused. Use a
# fresh cache dir per process.
os.environ["NEURON_COMPILE_CACHE_URL"] = tempfile.mkdtemp(prefix="neff_cache_")
# The axon cassette (compile record/replay) fingerprints the module interface
# but not the embedded Bass program; salt it per process so edits always
# recompile instead of replaying a stale executable.
os.environ["AXON_CASSETTE_SALT"] = f"ca-{os.getpid()}-{os.urandom(4).hex()}"

for _p in ("/opt/trn_rl_repo",):
    if os.path.isdir(_p) and _p not in sys.path:
        sys.path.insert(0, _p)

import numpy as np
from contextlib import ExitStack

import concourse.bass as bass
import concourse.tile as tile
from concourse import bacc, mybir
from concourse.bass_utils import run_bass_kernel_spmd

F32 = mybir.dt.float32
F32R = mybir.dt.float32r
F16 = mybir.dt.float16
AF = mybir.ActivationFunctionType
ALU = mybir.AluOpType

B, NQ, NK, D = 4, 2048, 2048, 512
H, DH = 8, 64
NQS = NQ // 2  # queries per core
TEMP = float(np.sqrt(512.0))
LN_EPS = 1e-5
N_CORES = 8

_CACHE = {}
# Interface salt: the remote executable cache fingerprints the module
# interface but not the embedded program; bump per kernel edit to force
# recompilation.
SALT = "v14"
SALT_N = 14
DEBUG = os.environ.get("CA_DEBUG", "0") == "1"


def _build_program():
    nc = bacc.Bacc("TRN2", target_bir_lowering=False, debug=False)

    def din(name, shape, dt=F32):
        return nc.dram_tensor(f"{name}_{SALT}", shape, dt,
                              kind="ExternalInput").ap()

    qt_d = din("qt", [128, 4, NQS], F16)
    kt_d = din("kt", [128, 4, NK], F16)
    vt_d = din("vt", [128, 4, NK], F16)
    wq_d = din("wq", [128, 4, D], F16)
    wk_d = din("wk", [128, 4, D], F16)
    wv_d = din("wv", [128, 4, D], F16)
    wo_d = din("wo", [128, 4, D], F32R)
    aq_d = din("aq", [2, D], F32R)
    ak_d = din("ak", [2, D], F32R)
    ao_d = din("ao", [2, D], F32R)
    gb_d = din("gb", [128, 2, 4], F32)
    grow_d = din("grow", [1, D], F32R)
    salt_d = din("salt", [1, 8 + SALT_N], F32)
    out_d = nc.dram_tensor(f"out_{SALT}", [128, 4, NQS], F32,
                           kind="ExternalOutput").ap()
    if DEBUG:
        def dout(name, shape, dt):
            return nc.dram_tensor(f"{name}_{SALT}", shape, dt,
                                  kind="ExternalOutput").ap()
        dbg_kts = dout("dbg_kts", [128, 4, NK], F16)
        dbg_qts = dout("dbg_qts", [128, 4, NQS], F16)
        dbg_vaug = dout("dbg_vaug", [128, 16, 8, 65], F16)
        dbg_ots = dout("dbg_ots", [128, 4, NQS], F32)
        dbg_rr = dout("dbg_rr", [64, 512], F32)

    with tile.TileContext(nc) as tc, ExitStack() as top:
        persist = top.enter_context(tc.tile_pool(name="persist", bufs=1))
        qTs = persist.tile([128, 4, NQS], F16)        # projected q
        kTs = persist.tile([128, 4, NK], F16)         # projected k
        vaug = persist.tile([128, 16, 8, 65], F16)    # v natural + ones col
        oTs = persist.tile([128, 4, NQS], F32R)       # attention out
        qtin = persist.tile([128, 4, NQS], F16)
        ktin = persist.tile([128, 4, NK], F16)
        vtin = persist.tile([128, 4, NK], F16)
        wq_sb = persist.tile([128, 4, D], F16)
        wk_sb = persist.tile([128, 4, D], F16)
        wv_sb = persist.tile([128, 4, D], F16)
        wo_sb = persist.tile([128, 4, D], F32R)
        aq_sb = persist.tile([2, D], F32R)
        ak_sb = persist.tile([2, D], F32R)
        ao_sb = persist.tile([2, D], F32R)
        gb_sb = persist.tile([128, 2, 4], F32)
        grow_sb = persist.tile([1, D], F32R)
        auxq = persist.tile([2, NQS], F32R)           # [m; std] rows for Q
        auxk = persist.tile([2, NK], F32R)
        auxo = persist.tile([2, NQS], F32R)           # [m*r; 1] rows, finale
        rstdQ = persist.tile([128, 2, 512], F32)      # replicated 1/std per
        rstdK = persist.tile([128, 4, 512], F32)      # 512-token chunk
        onesh = persist.tile([128, 128], F16)
        onesr = persist.tile([128, 128], F32R)
        eps_t = persist.tile([128, 1], F32)

        # Small latency-critical transfers first: the stats chains need
        # eps/weights immediately; anything queued behind the 5MB of input
        # DMAs stalls the whole prefix.
        nc.sync.dma_start(out=eps_t, in_=salt_d[0:1, 0:1].to_broadcast([128, 1]))
        nc.sync.dma_start(out=wk_sb, in_=wk_d)
        nc.sync.dma_start(out=wq_sb, in_=wq_d)
        nc.sync.dma_start(out=aq_sb, in_=aq_d)
        nc.sync.dma_start(out=ak_sb, in_=ak_d)
        for n2 in range(4):
            ns = slice(512 * n2, 512 * n2 + 512)
            nc.sync.dma_start(out=ktin[:, :, ns], in_=kt_d[:, :, ns])
        for n2 in range(2):
            ns = slice(512 * n2, 512 * n2 + 512)
            nc.sync.dma_start(out=qtin[:, :, ns], in_=qt_d[:, :, ns])
        nc.sync.dma_start(out=wv_sb, in_=wv_d)
        for n2 in range(4):
            ns = slice(512 * n2, 512 * n2 + 512)
            nc.sync.dma_start(out=vtin[:, :, ns], in_=vt_d[:, :, ns])
        nc.sync.dma_start(out=wo_sb, in_=wo_d)
        nc.sync.dma_start(out=ao_sb, in_=ao_d)
        nc.sync.dma_start(out=gb_sb, in_=gb_d)
        nc.sync.dma_start(out=grow_sb, in_=grow_d)
        nc.vector.memset(onesh, 1.0)
        # memset cannot target f32r; write through an f32 view (same bytes)
        nc.vector.memset(onesr.bitcast(F32), 1.0)
        nc.vector.memset(vaug[:, :, :, 64], 1.0)
        # row 0 is overwritten with m*r in the finale; row 1 stays all-ones
        nc.vector.memset(auxo.bitcast(F32), 1.0)

        pmm = top.enter_context(tc.tile_pool(name="pmm", bufs=1, space="PSUM"))
        work = top.enter_context(tc.tile_pool(name="work", bufs=1))
        # Prefix-only stats accumulators: separate banks so the stats chain
        # of chunk n+1 never waits on a projection accumulator and vice
        # versa. Closed before the attention pools open.
        pre_ctx = tc.tile_pool(name="pre", bufs=1, space="PSUM")
        pre = pre_ctx.__enter__()

        def ln_stats(xin, n2, aux, rstd_store, pool):
            """Column LN stats of token chunk n2 of xin [128, 4, *]; fills
            aux rows [m; std] and rstd_store[:, n2, :] (replicated)."""
            ns = slice(512 * n2, 512 * n2 + 512)
            ps_sum = pool.tile([128, 512], F32, name="st", bufs=4)
            for kc in range(4):
                nc.tensor.matmul(ps_sum, onesh, xin[:, kc, ns],
                                 start=(kc == 0), stop=(kc == 3))
            ps_ssq = pool.tile([128, 512], F32, name="st", bufs=4)
            for kc in range(4):
                sq = work.tile([128, 512], F16, name="sq", bufs=3)
                with nc.allow_low_precision("squares in fp16"):
                    nc.vector.tensor_mul(sq, xin[:, kc, ns], xin[:, kc, ns])
                nc.tensor.matmul(ps_ssq, onesh, sq,
                                 start=(kc == 0), stop=(kc == 3))
            m_b = work.tile([128, 512], F32, name="w32", bufs=6)
            nc.scalar.mul(m_b, ps_sum, 1.0 / 512.0)
            t2 = work.tile([128, 512], F32, name="w32", bufs=6)
            nc.vector.tensor_mul(t2, m_b, ps_sum)
            dv = work.tile([128, 512], F32, name="w32", bufs=6)
            nc.vector.tensor_sub(dv, ps_ssq, t2)
            std_b = work.tile([128, 512], F32, name="w32", bufs=6)
            nc.scalar.activation(std_b, dv, AF.Sqrt, bias=eps_t,
                                 scale=1.0 / 512.0)
            nc.vector.reciprocal_approx_fast(rstd_store[:, n2, :], std_b)
            # engine ops cannot move data across partitions; DMA the stat
            # rows into the K=2 aux operand instead.
            nc.sync.dma_start(out=aux[0:1, ns].bitcast(F32), in_=m_b[0:1, :])
            nc.sync.dma_start(out=aux[1:2, ns].bitcast(F32), in_=std_b[0:1, :])

        def proj_chunk(dst, xin, w_sb, a_sb, aux, rstd_store, jc, n2):
            """dst[:, jc, ns] = (sum_kc w'[:,kc,js].T @ x[:,kc,ns] + aux)*r."""
            ns = slice(512 * n2, 512 * n2 + 512)
            js = slice(128 * jc, 128 * jc + 128)
            pg = pmm.tile([128, 512], F32, name="pmm", bufs=2)
            for kc in range(4):
                nc.tensor.matmul(pg, w_sb[:, kc, js], xin[:, kc, ns],
                                 start=(kc == 0), stop=False)
            nc.tensor.matmul(pg, a_sb[:, js], aux[:, ns],
                             start=False, stop=True)
            with nc.allow_low_precision("fp16 activations"):
                nc.vector.tensor_mul(dst[:, jc, ns], pg, rstd_store[:, n2, :])

        # ---- Prefix: K stats + K proj jc0, V proj, Q stats + Q proj jc0 ----
        for n2 in range(4):
            ln_stats(ktin, n2, auxk, rstdK, pre)
        for n2 in range(4):
            proj_chunk(kTs, ktin, wk_sb, ak_sb, auxk, rstdK, 0, n2)
        for t in range(16):
            ts = slice(128 * t, 128 * t + 128)
            pv = pmm.tile([128, 512], F32, name="pmm", bufs=2)
            for kc in range(4):
                nc.tensor.matmul(pv, vtin[:, kc, ts], wv_sb[:, kc, :],
                                 start=(kc == 0), stop=(kc == 3))
            with nc.allow_low_precision("fp16 activations"):
                nc.scalar.copy(
                    vaug[:, t, :, 0:64],
                    pv.rearrange("p (h v) -> p h v", h=8))
        for n2 in range(2):
            ln_stats(qtin, n2, auxq, rstdQ, pre)
        for n2 in range(2):
            proj_chunk(qTs, qtin, wq_sb, aq_sb, auxq, rstdQ, 0, n2)
        pre_ctx.__exit__(None, None, None)  # release stats banks for psL

        # Remaining projection chunks stream through the attention window
        # (the exp stream on ACT is the bottleneck there; PE has slack).
        tasks = {}

        def add_task(u, fn):
            tasks.setdefault(u, []).append(fn)

        def mk_proj(dst, xin, w_sb, a_sb, aux, rstd_store, jc, n2):
            return lambda: proj_chunk(dst, xin, w_sb, a_sb, aux, rstd_store,
                                      jc, n2)

        for jc in range(1, 4):
            base = 16 + 28 * (jc - 1)
            for n2 in range(4):
                add_task(base + 2 * n2, mk_proj(kTs, ktin, wk_sb, ak_sb,
                                                auxk, rstdK, jc, n2))
            for n2 in range(2):
                add_task(base + 10 + 3 * n2, mk_proj(qTs, qtin, wq_sb, aq_sb,
                                                     auxq, rstdQ, jc, n2))

        # ---- Attention: units (P, c, p); c-outer so one (P,c) pass holds
        # only 2 O-accumulator banks; psL double-buffered. ----
        at_psL = top.enter_context(tc.tile_pool(name="at_psL", bufs=1,
                                                space="PSUM"))
        at_po = top.enter_context(tc.tile_pool(name="at_po", bufs=1,
                                               space="PSUM"))
        at_sb = top.enter_context(tc.tile_pool(name="at_sb", bufs=1))

        units = [(P, c, p) for P in range(4) for c in range(2)
                 for p in range(16)]
        LAG = 4
        pend = {}   # unit idx -> (P, c, p, psO pair, ex tile)

        def emit_O(u):
            P, c, p, ps_o, ex = pend.pop(u)
            for hh in range(2):
                nc.tensor.matmul(ps_o[hh], vaug[:, p, 2 * P + hh, :],
                                 ex[:, 512 * hh: 512 * hh + 512],
                                 start=(p == 0), stop=(p == 15))
            if p == 15:
                cs = slice(512 * c, 512 * c + 512)
                for hh in range(2):
                    # Evacuate the accumulator bank with two quick aligned
                    # DVE copies so the next (P,c) group's first O-matmul
                    # isn't blocked behind the whole normalize chain.
                    den = at_sb.tile([65, 512], F32, name="rr", bufs=2)
                    nc.vector.tensor_copy(den[64:65, :], ps_o[hh][64:65, :])
                    nst = at_sb.tile([64, 512], F32, name="nst", bufs=2)
                    nc.vector.tensor_copy(nst, ps_o[hh][0:64, :])
                    # recip_approx only works from SBUF at partition 0: DMA
                    # the den row down, then reciprocal + broadcast.
                    den0 = at_sb.tile([1, 512], F32, name="tl32", bufs=6)
                    nc.sync.dma_start(out=den0, in_=den[64:65, :])
                    rr0 = at_sb.tile([1, 512], F32, name="tl32", bufs=6)
                    nc.vector.reciprocal_approx_fast(rr0, den0)
                    rrb = at_sb.tile([64, 512], F32, name="tl32", bufs=6)
                    nc.gpsimd.partition_broadcast(rrb, rr0)
                    rb = 64 * hh
                    if hh == 0:
                        # rows align with oTs: write the normalized block
                        # in place, no DMA hop.
                        with nc.allow_low_precision("f32r storage"):
                            nc.vector.tensor_mul(oTs[0:64, P, cs], nst, rrb)
                    else:
                        ost = at_sb.tile([64, 512], F32, name="tl32", bufs=6)
                        nc.vector.tensor_mul(ost, nst, rrb)
                        nc.sync.dma_start(
                            out=oTs[rb:rb + 64, P, cs].bitcast(F32), in_=ost)
                    if DEBUG and P == 0 and c == 0 and hh == 0:
                        nc.sync.dma_start(out=dbg_rr, in_=rrb)

        # ---- Finale: LN fold (prescale form) + Wo + gelu + residual.
        # The n2=0 half of the stats (matmuls + DVE chain, no Sqrt — a Sqrt
        # would thrash the ACT table set mid-exp-stream) is emitted as a
        # task inside the attention window once all c=0 groups are done. ----
        fin = {}

        def fin_early(n2):
            ns = slice(512 * n2, 512 * n2 + 512)
            ps_sum = pmm.tile([128, 512], F32, name="pmm", bufs=2)
            for jc in range(4):
                nc.tensor.matmul(ps_sum, onesr, oTs[:, jc, ns],
                                 start=(jc == 0), stop=(jc == 3))
            ps_ssq = pmm.tile([128, 512], F32, name="pmm", bufs=2)
            for jc in range(4):
                sqo = work.tile([128, 512], F32R, name="w32", bufs=6)
                with nc.allow_low_precision("f32r keeps fp32 storage"):
                    nc.vector.tensor_mul(sqo, oTs[:, jc, ns], oTs[:, jc, ns])
                nc.tensor.matmul(ps_ssq, onesr, sqo,
                                 start=(jc == 0), stop=(jc == 3))
            m_b = work.tile([128, 512], F32, name="w32", bufs=6)
            nc.scalar.mul(m_b, ps_sum, 1.0 / 512.0)
            t2 = work.tile([128, 512], F32, name="w32", bufs=6)
            nc.vector.tensor_mul(t2, m_b, ps_sum)
            dv = work.tile([128, 512], F32, name="w32", bufs=6)
            nc.vector.tensor_sub(dv, ps_ssq, t2)
            fin[n2] = (m_b, dv)

        def fin_late(n2):
            ns = slice(512 * n2, 512 * n2 + 512)
            m_b, dv = fin[n2]
            std_b = work.tile([128, 512], F32, name="w32", bufs=6)
            nc.scalar.activation(std_b, dv, AF.Sqrt, bias=eps_t,
                                 scale=1.0 / 512.0)
            r_b = work.tile([128, 512], F32, name="w32", bufs=6)
            nc.vector.reciprocal_approx_fast(r_b, std_b)
            with nc.allow_low_precision("f32r keeps fp32 storage"):
                nc.vector.tensor_mul(auxo[0:1, ns], m_b[0:1, :], r_b[0:1, :])
                for jc in range(4):
                    nc.vector.tensor_mul(oTs[:, jc, ns], oTs[:, jc, ns], r_b)

        def fin_wo(jc, n2):
            js = slice(128 * jc, 128 * jc + 128)
            ns = slice(512 * n2, 512 * n2 + 512)
            pg = pmm.tile([128, 512], F32, name="pmm", bufs=2)
            for kc in range(4):
                nc.tensor.matmul(pg, wo_sb[:, kc, js], oTs[:, kc, ns],
                                 start=(kc == 0), stop=False)
            nc.tensor.matmul(pg, ao_sb[:, js], auxo[:, ns],
                             start=False, stop=True)
            pbm = pmm.tile([128, 512], F32, name="pmm", bufs=2)
            nc.tensor.matmul(pbm, grow_sb[0:1, js], auxo[0:1, ns],
                             start=True, stop=True)
            gl = work.tile([128, 512], F32, name="w32", bufs=6)
            nc.scalar.activation(gl, pg, AF.Gelu)
            u2 = work.tile([128, 512], F32, name="w32", bufs=6)
            # u2 = oTs_scaled*g - m*r*g   (oTs already prescaled by r)
            nc.vector.scalar_tensor_tensor(
                u2, oTs[:, jc, ns], gb_sb[:, 0, jc:jc + 1], pbm,
                op0=ALU.mult, op1=ALU.subtract)
            of = work.tile([128, 512], F32, name="w32", bufs=6)
            nc.vector.scalar_tensor_tensor(
                of, u2, gb_sb[:, 1, jc:jc + 1], gl,
                op0=ALU.add, op1=ALU.add)
            nc.sync.dma_start(out=out_d[:, jc, ns], in_=of)

        add_task(120, lambda: fin_early(0))

        ps_o_cur = None
        for u, (P, c, p) in enumerate(units):
            if p == 0:
                ps_o_cur = [at_po.tile([65, 512], F32, name=f"po{hh}",
                                       bufs=1) for hh in range(2)]
            psL = at_psL.tile([128, 1024], F32, name="psL", bufs=2)
            ks = slice(128 * p, 128 * p + 128)
            cs = slice(512 * c, 512 * c + 512)
            for hh in range(2):
                rb = 64 * hh
                nc.tensor.matmul(psL[:, 512 * hh: 512 * hh + 512],
                                 kTs[rb:rb + 64, P, ks],
                                 qTs[rb:rb + 64, P, cs],
                                 start=True, stop=True)
            ex = at_sb.tile([128, 1024], F16, name="ex", bufs=LAG + 1)
            nc.scalar.activation(ex, psL, AF.Exp, scale=1.0 / TEMP)
            pend[u] = (P, c, p, ps_o_cur, ex)
            if u >= LAG:
                emit_O(u - LAG)
            for fn in tasks.pop(u, ()):
                fn()
        for u in range(len(units) - LAG, len(units)):
            emit_O(u)

        if DEBUG:
            nc.sync.dma_start(out=dbg_kts, in_=kTs)
            nc.sync.dma_start(out=dbg_qts, in_=qTs)
            nc.sync.dma_start(out=dbg_vaug, in_=vaug)
            nc.sync.dma_start(out=dbg_ots, in_=oTs.bitcast(F32))

        fin_late(0)
        for jc in range(4):
            fin_wo(jc, 0)
        fin_early(1)
        fin_late(1)
        for jc in range(4):
            fin_wo(jc, 1)

    nc.compile()
    return nc


def _chunk_fm(x):
    """[512, N] feature-major -> [128, 4, N] (partition, chunk, col)."""
    n = x.shape[1]
    return np.ascontiguousarray(x.reshape(4, 128, n).transpose(1, 0, 2))


def _prep_inputs(Q, K, V, Wq, Wk, Wv, Wo, g, b, go, bo):
    WqT = np.ascontiguousarray((Wq * g[None, :]).T)
    WkT = np.ascontiguousarray((Wk * g[None, :]).T)
    WvT = np.ascontiguousarray(Wv.T)
    WoT = np.ascontiguousarray((Wo * go[None, :]).T)
    f16 = np.float16
    shared = {
        f"wq_{SALT}": _chunk_fm(WqT).astype(f16),
        f"wk_{SALT}": _chunk_fm(WkT).astype(f16),
        f"wv_{SALT}": _chunk_fm(WvT).astype(f16),
        f"wo_{SALT}": _chunk_fm(WoT),
        f"aq_{SALT}": np.ascontiguousarray(np.stack([-WqT.sum(0), Wq @ b])),
        f"ak_{SALT}": np.ascontiguousarray(np.stack([-WkT.sum(0), Wk @ b])),
        f"ao_{SALT}": np.ascontiguousarray(np.stack([-WoT.sum(0), Wo @ bo])),
        f"gb_{SALT}": np.ascontiguousarray(
            np.stack([go.reshape(4, 128).T, bo.reshape(4, 128).T], axis=1)),
        f"grow_{SALT}": np.ascontiguousarray(go[None, :]),
    }
    in_maps = []
    for core in range(N_CORES):
        bi, half = core // 2, core % 2
        qs = slice(half * NQS, (half + 1) * NQS)
        m = dict(shared)
        m[f"salt_{SALT}"] = np.full((1, 8 + SALT_N), LN_EPS, np.float32)
        m[f"qt_{SALT}"] = _chunk_fm(np.ascontiguousarray(Q[bi, qs, :].T)).astype(f16)
        m[f"kt_{SALT}"] = _chunk_fm(np.ascontiguousarray(K[bi].T)).astype(f16)
        m[f"vt_{SALT}"] = _chunk_fm(np.ascontiguousarray(V[bi].T)).astype(f16)
        in_maps.append(m)
    return in_maps


def kernel(Q, K, V, Wq, Wk, Wv, Wo, ln_qk_g, ln_qk_b, ln_o_g, ln_o_b,
           _trace=False):
    args = [np.asarray(a, dtype=np.float32) for a in
            (Q, K, V, Wq, Wk, Wv, Wo, ln_qk_g, ln_qk_b, ln_o_g, ln_o_b)]
    if "nc" not in _CACHE:
        _CACHE["nc"] = _build_program()
    nc = _CACHE["nc"]
    in_maps = _prep_inputs(*args)
    res = run_bass_kernel_spmd(nc, in_maps, core_ids=list(range(N_CORES)),
                               trace=_trace)
    _CACHE["last_results"] = res
    out = np.empty((B, NQ, D), dtype=np.float32)
    for core in range(N_CORES):
        bi, half = core // 2, core % 2
        o = res.results[core][f"out_{SALT}"]  # [128, 4, NQS]
        out[bi, half * NQS : (half + 1) * NQS, :] = (
            o.transpose(1, 0, 2).reshape(D, NQS).T)
    return out
